# revision 21
# baseline (speedup 1.0000x reference)
"""Trainium2 Bass kernel for nn_DKOKernel (dense pairwise MLP + PSD head).

Math (per batch b):
  hx[f,i] = sum_c wx[f,c] x[b,i,c];  hy[f,j] = sum_c wy[f,c] y[b,j,c]
  h1 = relu(bn1(hx_i + hy_j + b1))          (512)
  h2 = relu(bn2(W2 h1 + b2))                (256)
  h3 = relu(bn3(W3 h2 + b3))                (128)
  e  = W4 h3 + b4                           (64)
  s[b,i,:] = sum_j e[b,i,j,:]
  out[b,i,j] = sum_k e[b,i,j,k] * s[b,i,k]

Sharding: data-parallel over batch — core b handles batch b (B=8 = n_cores).
All BatchNorm affine factors are folded into the weights/biases on the host.

Head algebra (avoids materializing e):
  q_i  = sum_j h3_ij                    (per-i, via free-dim reduce)
  s_i  = W4 q_i + ny*b4                 (tiny N=C matmul, fp32)
  v_i  = W4^T s_i                       (tiny N=C matmul, fp32)
  c_i  = wc.q_i + ny*|b4|^2             (wc = W4^T b4, tiny N=C matmul, fp32)
  out[i,j] = h3_ij . v_i + c_i          (elementwise mul + ones-matmul + add)

Device-side layout: features on partitions, (i-block, j) pairs on the free
dim. C=4 i-rows per chunk => free dim 512 per matmul (fp32 PSUM bank max and
the ap_size>=256 threshold where fp32r matmuls run 1 cycle/row).

The big matmuls (L2, L3, ones-reduce) run in float32r (TF32-like, ~1.6e-4
relative per matmul, 4x faster than fp32). Their operands must be produced
as float32r: the weights are DMA-loaded into f32r tiles, the activations are
written as f32r by the producing DVE/Pool/ACT ops.
"""

import numpy as np
from contextlib import ExitStack

import concourse.bacc as bacc
import concourse.tile as tile
from concourse import mybir
from concourse.bass_utils import run_bass_kernel_spmd

F32 = mybir.dt.float32
F32R = mybir.dt.float32r
AF = mybir.ActivationFunctionType
ALU = mybir.AluOpType
AX = mybir.AxisListType

EPS = 1e-5
B = 8
N = 128          # nx == ny
F = 128          # input feature dim
D1, D2, D3, D4 = 512, 256, 128, 64
C = 4            # i-rows per chunk -> free dim C*N = 512
import os
NCH = int(os.environ.get('NCH_OVERRIDE', N // C))
SKIP_HEAD = os.environ.get('SKIP_HEAD', '0') == '1'
SKIP_SMALL_DMA = os.environ.get('SKIP_SMALL_DMA', '0') == '1'
SKIP_SETUP = os.environ.get('SKIP_SETUP', '0') == '1'
SKIP_MLP = os.environ.get('SKIP_MLP', '0') == '1'
REPEAT = int(os.environ.get('REPEAT', '1'))

USE_F32R = True
MMDT = F32R if USE_F32R else F32
# Which of the 16 L1 ops per chunk run on DVE (rest go to GPSIMD/Pool).
L1_DVE = frozenset({0, 3, 6, 9, 12})


def build_module(repeat=None):
    rep = REPEAT if repeat is None else repeat
    nc = bacc.Bacc()

    xT = nc.declare_dram_parameter("xT", [F, N], F32, isOutput=False)
    yT = nc.declare_dram_parameter("yT", [F, N], F32, isOutput=False)
    wxT = nc.declare_dram_parameter("wxT", [F, D1], F32, isOutput=False)
    wyT = nc.declare_dram_parameter("wyT", [F, D1], F32, isOutput=False)
    w2T = nc.declare_dram_parameter("w2T", [4, 128, D2], F32, isOutput=False)
    w3T = nc.declare_dram_parameter("w3T", [2, 128, D3], F32, isOutput=False)
    w4T = nc.declare_dram_parameter("w4T", [128, D4], F32, isOutput=False)
    w4N = nc.declare_dram_parameter("w4N", [D4, 128], F32, isOutput=False)
    wcd = nc.declare_dram_parameter("wc", [128], F32, isOutput=False)
    c1d = nc.declare_dram_parameter("c1", [4, 128], F32, isOutput=False)
    c2d = nc.declare_dram_parameter("c2", [2, 128], F32, isOutput=False)
    c3d = nc.declare_dram_parameter("c3", [128], F32, isOutput=False)
    b4x = nc.declare_dram_parameter("b4x", [D4], F32, isOutput=False)
    c0d = nc.declare_dram_parameter("c0", [1], F32, isOutput=False)
    onesd = nc.declare_dram_parameter("ones", [128], F32, isOutput=False)
    out_d = nc.declare_dram_parameter("out", [N, N], F32, isOutput=True)

    with tile.TileContext(nc) as tc:
        with ExitStack() as ctx:
            singles = ctx.enter_context(tc.tile_pool(name="singles", bufs=1))

            xT_s = singles.tile([F, N], F32)
            yT_s = singles.tile([F, N], F32)
            wxT_s = singles.tile([F, D1], F32)
            wyT_s = singles.tile([F, D1], F32)
            w2_f = singles.tile([128, 4, D2], F32)
            w3_f = singles.tile([128, 2, D3], F32)
            w2_s = singles.tile([128, 4, D2], MMDT)
            w3_s = singles.tile([128, 2, D3], MMDT)
            w4_s = singles.tile([128, D4], F32)
            w4N_s = singles.tile([D4, 128], F32)
            wc_s = singles.tile([128, 1], F32)
            c1_s = singles.tile([128, 4], F32)
            c2_s = singles.tile([128, 2], F32)
            c3_s = singles.tile([128, 1], F32)
            b4x_s = singles.tile([D4, 1], F32)
            c0_s = singles.tile([1, 1], F32)
            ones_f = singles.tile([128, 1], F32)
            ones_s = singles.tile([128, 1], MMDT)
            hx_s = singles.tile([128, 4, N], F32)
            hy_s = singles.tile([128, 4, N], F32)
            out_s = singles.tile([1, N * N], F32)

            nc.sync.dma_start(out=xT_s, in_=xT[:, :])
            nc.sync.dma_start(out=yT_s, in_=yT[:, :])
            nc.sync.dma_start(out=wxT_s, in_=wxT[:, :])
            nc.sync.dma_start(out=wyT_s, in_=wyT[:, :])
            for kc in range(4):
                nc.sync.dma_start(out=w2_f[:, kc, :], in_=w2T[kc, :, :])
            for kc in range(2):
                nc.sync.dma_start(out=w3_f[:, kc, :], in_=w3T[kc, :, :])
            nc.sync.dma_start(out=w4_s, in_=w4T[:, :])
            nc.sync.dma_start(out=w4N_s, in_=w4N[:, :])
            if SKIP_SMALL_DMA:
                for tl in (wc_s, c1_s, c2_s, c3_s, b4x_s, c0_s, ones_f):
                    nc.vector.memset(tl, 0.01)
            else:
                nc.sync.dma_start(out=wc_s[:, 0], in_=wcd[:])
                for fc in range(4):
                    nc.sync.dma_start(out=c1_s[:, fc], in_=c1d[fc, :])
                for mc in range(2):
                    nc.sync.dma_start(out=c2_s[:, mc], in_=c2d[mc, :])
                nc.sync.dma_start(out=c3_s[:, 0], in_=c3d[:])
                nc.sync.dma_start(out=b4x_s[:, 0], in_=b4x[:])
                nc.sync.dma_start(out=c0_s[0, :], in_=c0d[:])
                nc.sync.dma_start(out=ones_f[:, 0], in_=onesd[:])
            if USE_F32R:
                nc.vector.tensor_copy(out=w2_s, in_=w2_f)
                nc.vector.tensor_copy(out=w3_s, in_=w3_f)
                nc.vector.tensor_copy(out=ones_s, in_=ones_f)
            else:
                w2_s, w3_s, ones_s = w2_f, w3_f, ones_f

            # hx[f,i] (k1 folded into wx on host), hy''[f,j] (+c1).
            # Setup matmuls run in plain fp32 (tiny, and xT/yT arrive as f32).
            if SKIP_SETUP:
                nc.vector.memset(hx_s, 0.02)
                nc.vector.memset(hy_s, 0.03)
            elif True:
              with tc.tile_pool(name="psum_setup", bufs=2, space="PSUM") as pp:
                for fc in range(4):
                    ph = pp.tile([128, N], F32, tag="ph")
                    nc.tensor.matmul(
                        ph, lhsT=wxT_s[:, fc * 128:(fc + 1) * 128],
                        rhs=xT_s, start=True, stop=True)
                    nc.scalar.activation(hx_s[:, fc, :], ph, AF.Copy)
                    py_ = pp.tile([128, N], F32, tag="ph")
                    nc.tensor.matmul(
                        py_, lhsT=wyT_s[:, fc * 128:(fc + 1) * 128],
                        rhs=yT_s, start=True, stop=True)
                    nc.scalar.activation(hy_s[:, fc, :], py_, AF.Identity,
                                         bias=c1_s[:, fc:fc + 1])


            work = ctx.enter_context(tc.tile_pool(name="work", bufs=3))
            psum = ctx.enter_context(tc.tile_pool(name="psum", bufs=2,
                                                  space="PSUM"))
            psum1 = ctx.enter_context(tc.tile_pool(name="psum1", bufs=1,
                                                   space="PSUM"))

            import contextlib
            loop_cm = (tc.For_i(0, rep, 1) if rep > 1
                       else contextlib.nullcontext())
            with loop_cm:
              for t in range(NCH):
                # ---- L1: h1[f, (ii,j)] = relu(hx[f, C*t+ii] + hy[f, j]) ----
                # One fused tensor_scalar (add, then max 0) per (fc, ii);
                # the per-partition scalar is the hx column. Split between
                # DVE (2x fp32 mode) and Pool for engine balance.
                h1 = work.tile([128, 4, C * N], MMDT, tag="h1")
                for fc in range(4):
                    for ii in range(C):
                        idx = fc * C + ii
                        eng = nc.vector if idx in L1_DVE else nc.gpsimd
                        eng.tensor_scalar(
                            out=h1[:, fc, ii * N:(ii + 1) * N],
                            in0=hy_s[:, fc, :],
                            scalar1=hx_s[:, fc, C * t + ii:C * t + ii + 1],
                            scalar2=0.0,
                            op0=ALU.add, op1=ALU.max)

                if SKIP_MLP:
                    nc.vector.tensor_copy(
                        out=out_s[0:1, C * N * t:C * N * (t + 1)],
                        in_=h1[0:1, 0, :].bitcast(F32))
                    continue
                # ---- L2: h2 = relu(W2' h1 + c2) ----
                h2 = work.tile([128, 2, C * N], MMDT, tag="h2")
                for mc in range(2):
                    p2 = psum.tile([128, C * N], F32, tag="p2")
                    for kc in range(4):
                        nc.tensor.matmul(
                            p2,
                            lhsT=w2_s[:, kc, mc * 128:(mc + 1) * 128],
                            rhs=h1[:, kc, :],
                            start=(kc == 0), stop=(kc == 3))
                    nc.scalar.activation(h2[:, mc, :], p2, AF.Relu,
                                         bias=c2_s[:, mc:mc + 1])

                # ---- L3: h3 = relu(W3' h2 + c3) ----
                h3 = work.tile([128, C * N], MMDT, tag="h3")
                p3 = psum.tile([128, C * N], F32, tag="p3")
                for kc in range(2):
                    nc.tensor.matmul(
                        p3, lhsT=w3_s[:, kc, :], rhs=h2[:, kc, :],
                        start=(kc == 0), stop=(kc == 1))
                nc.scalar.activation(h3, p3, AF.Relu, bias=c3_s[:, 0:1])
                h3f = h3.bitcast(F32)

                # ---- head ----
                if SKIP_HEAD:
                    nc.vector.tensor_copy(
                        out=out_s[0:1, C * N * t:C * N * (t + 1)],
                        in_=h3f[0:1, :])
                    continue
                q = work.tile([128, C], F32, tag="q")
                nc.vector.tensor_reduce(
                    out=q, in_=h3f.rearrange("p (a b) -> p a b", a=C),
                    axis=AX.X, op=ALU.add)

                ps_s = psum1.tile([D4, C], F32, tag="ss")
                nc.tensor.matmul(ps_s, lhsT=w4_s, rhs=q,
                                 start=True, stop=True)
                s_sb = work.tile([D4, C], F32, tag="s")
                nc.scalar.activation(s_sb, ps_s, AF.Identity, bias=b4x_s)

                ps_v = psum1.tile([128, C], F32, tag="sv")
                nc.tensor.matmul(ps_v, lhsT=w4N_s, rhs=s_sb,
                                 start=True, stop=True)
                v_sb = work.tile([128, C], F32, tag="v")
                nc.scalar.activation(v_sb, ps_v, AF.Copy)

                ps_c = psum1.tile([1, C], F32, tag="sc")
                nc.tensor.matmul(ps_c, lhsT=wc_s, rhs=q,
                                 start=True, stop=True)
                c_sb = work.tile([1, C], F32, tag="c")
                nc.scalar.activation(c_sb, ps_c, AF.Identity,
                                     bias=c0_s[0:1, 0:1])

                em = work.tile([128, C * N], MMDT, tag="em")
                for ii in range(C):
                    nc.vector.tensor_scalar(
                        out=em[:, ii * N:(ii + 1) * N],
                        in0=h3f[:, ii * N:(ii + 1) * N],
                        scalar1=v_sb[:, ii:ii + 1],
                        scalar2=None,
                        op0=ALU.mult)

                pf = psum1.tile([1, C * N], F32, tag="pf")
                nc.tensor.matmul(pf, lhsT=ones_s, rhs=em,
                                 start=True, stop=True)
                nc.vector.tensor_tensor(
                    out=out_s[0:1, C * N * t:C * N * (t + 1)].rearrange(
                        "p (a b) -> p a b", a=C),
                    in0=pf.rearrange("p (a b) -> p a b", a=C),
                    in1=c_sb.unsqueeze(2).broadcast_to([1, C, N]),
                    op=ALU.add)

            nc.sync.dma_start(
                out=out_d[:, :].unsqueeze(0),
                in_=out_s[0:1, :].rearrange("p (a b) -> p a b", a=N))
    nc.finalize()
    return nc


_NC_CACHE = None


def _get_nc():
    global _NC_CACHE
    if _NC_CACHE is None:
        _NC_CACHE = build_module()
    return _NC_CACHE


def host_prep(inputs):
    """Fold the BatchNorm affines into weights/biases; pre-transpose
    everything into the device layouts. Returns the per-core input maps."""
    f32 = np.float32
    x = np.asarray(inputs["x"], f32)
    y = np.asarray(inputs["y"], f32)
    w1, b1 = np.asarray(inputs["w1"], f32), np.asarray(inputs["b1"], f32)
    w2, b2 = np.asarray(inputs["w2"], f32), np.asarray(inputs["b2"], f32)
    w3, b3 = np.asarray(inputs["w3"], f32), np.asarray(inputs["b3"], f32)
    w4, b4 = np.asarray(inputs["w4"], f32), np.asarray(inputs["b4"], f32)

    k1 = inputs["g1"] / np.sqrt(inputs["v1"] + EPS)
    c1 = k1 * (b1 - inputs["m1"]) + inputs["be1"]
    k2 = inputs["g2"] / np.sqrt(inputs["v2"] + EPS)
    c2 = k2 * (b2 - inputs["m2"]) + inputs["be2"]
    k3 = inputs["g3"] / np.sqrt(inputs["v3"] + EPS)
    c3 = k3 * (b3 - inputs["m3"]) + inputs["be3"]

    wx = w1[:, :F] * k1[:, None]          # (512, 128)
    wy = w1[:, F:] * k1[:, None]
    w2f = w2 * k2[:, None]                # (256, 512)
    w3f = w3 * k3[:, None]                # (128, 256)

    shared = {
        "wxT": np.ascontiguousarray(wx.T, f32),                    # (128, 512)
        "wyT": np.ascontiguousarray(wy.T, f32),
        "w2T": np.ascontiguousarray(w2f.T.reshape(4, 128, D2), f32),
        "w3T": np.ascontiguousarray(w3f.T.reshape(2, 128, D3), f32),
        "w4T": np.ascontiguousarray(w4.T, f32),                    # (128, 64)
        "w4N": np.ascontiguousarray(w4, f32),                      # (64, 128)
        "wc": np.ascontiguousarray(w4.T @ b4, f32),                # (128,)
        "c1": np.ascontiguousarray(c1.reshape(4, 128), f32),
        "c2": np.ascontiguousarray(c2.reshape(2, 128), f32),
        "c3": np.ascontiguousarray(c3, f32),
        "b4x": np.ascontiguousarray(N * b4, f32),                  # ny*b4
        "c0": np.asarray([N * float(b4 @ b4)], f32),               # ny*|b4|^2
        "ones": np.ones(128, f32),
    }
    in_maps = []
    for b in range(B):
        m = dict(shared)
        m["xT"] = np.ascontiguousarray(x[b].T, f32)
        m["yT"] = np.ascontiguousarray(y[b].T, f32)
        in_maps.append(m)
    return in_maps


def kernel(**inputs):
    nc = _get_nc()
    in_maps = host_prep(inputs)
    res = run_bass_kernel_spmd(nc, in_maps, list(range(B)))
    out = np.stack([res.results[b]["out"] for b in range(B)], axis=0)
    return out.astype(np.float32)


# revision 22
# speedup vs baseline: 1.0245x; 1.0245x over previous
"""Trainium2 Bass kernel for nn_DKOKernel (dense pairwise MLP + PSD head).

Math (per batch b):
  hx[f,i] = sum_c wx[f,c] x[b,i,c];  hy[f,j] = sum_c wy[f,c] y[b,j,c]
  h1 = relu(bn1(hx_i + hy_j + b1))          (512)
  h2 = relu(bn2(W2 h1 + b2))                (256)
  h3 = relu(bn3(W3 h2 + b3))                (128)
  e  = W4 h3 + b4                           (64)
  s[b,i,:] = sum_j e[b,i,j,:]
  out[b,i,j] = sum_k e[b,i,j,k] * s[b,i,k]

Sharding: data-parallel over batch — core b handles batch b (B=8 = n_cores).
All BatchNorm affine factors are folded into the weights/biases on the host.

Head algebra (avoids materializing e):
  q_i  = sum_j h3_ij                    (per-i, via free-dim reduce)
  s_i  = W4 q_i + ny*b4                 (tiny N=C matmul, fp32)
  v_i  = W4^T s_i                       (tiny N=C matmul, fp32)
  c_i  = wc.q_i + ny*|b4|^2             (wc = W4^T b4, tiny N=C matmul, fp32)
  out[i,j] = h3_ij . v_i + c_i          (elementwise mul + ones-matmul + add)

Device-side layout: features on partitions, (i-block, j) pairs on the free
dim. C=4 i-rows per chunk => free dim 512 per matmul (fp32 PSUM bank max and
the ap_size>=256 threshold where fp32r matmuls run 1 cycle/row).

The big matmuls (L2, L3, ones-reduce) run in float32r (TF32-like, ~1.6e-4
relative per matmul, 4x faster than fp32). Their operands must be produced
as float32r: the weights are DMA-loaded into f32r tiles, the activations are
written as f32r by the producing DVE/Pool/ACT ops.
"""

import numpy as np
from contextlib import ExitStack

import concourse.bacc as bacc
import concourse.tile as tile
from concourse import mybir
from concourse.bass_utils import run_bass_kernel_spmd

F32 = mybir.dt.float32
F32R = mybir.dt.float32r
AF = mybir.ActivationFunctionType
ALU = mybir.AluOpType
AX = mybir.AxisListType

EPS = 1e-5
B = 8
N = 128          # nx == ny
F = 128          # input feature dim
D1, D2, D3, D4 = 512, 256, 128, 64
C = 4            # i-rows per chunk -> free dim C*N = 512
import os
NCH = int(os.environ.get('NCH_OVERRIDE', N // C))
SKIP_HEAD = os.environ.get('SKIP_HEAD', '0') == '1'
SKIP_SMALL_DMA = os.environ.get('SKIP_SMALL_DMA', '0') == '1'
SKIP_SETUP = os.environ.get('SKIP_SETUP', '0') == '1'
SKIP_MLP = os.environ.get('SKIP_MLP', '0') == '1'
REPEAT = int(os.environ.get('REPEAT', '1'))

USE_F32R = True
MMDT = F32R if USE_F32R else F32
# Which of the 16 L1 ops per chunk run on DVE (rest go to GPSIMD/Pool).
L1_DVE = frozenset({0, 3, 6, 9, 12})


def build_module(repeat=None, skip_l1=False, skip_mlp=None,
                 skip_head=None, l1_alldve=False):
    rep = REPEAT if repeat is None else repeat
    _skip_mlp = SKIP_MLP if skip_mlp is None else skip_mlp
    _skip_head = SKIP_HEAD if skip_head is None else skip_head
    _l1_dve = frozenset(range(16)) if l1_alldve else L1_DVE
    nc = bacc.Bacc()

    xT = nc.declare_dram_parameter("xT", [F, N], F32, isOutput=False)
    yT = nc.declare_dram_parameter("yT", [F, N], F32, isOutput=False)
    wxT = nc.declare_dram_parameter("wxT", [F, D1], F32, isOutput=False)
    wyT = nc.declare_dram_parameter("wyT", [F, D1], F32, isOutput=False)
    w2T = nc.declare_dram_parameter("w2T", [4, 128, D2], F32, isOutput=False)
    w3T = nc.declare_dram_parameter("w3T", [2, 128, D3], F32, isOutput=False)
    w4T = nc.declare_dram_parameter("w4T", [128, D4], F32, isOutput=False)
    w4N = nc.declare_dram_parameter("w4N", [D4, 128], F32, isOutput=False)
    wcd = nc.declare_dram_parameter("wc", [128], F32, isOutput=False)
    c1d = nc.declare_dram_parameter("c1", [4, 128], F32, isOutput=False)
    c2d = nc.declare_dram_parameter("c2", [2, 128], F32, isOutput=False)
    c3d = nc.declare_dram_parameter("c3", [128], F32, isOutput=False)
    b4x = nc.declare_dram_parameter("b4x", [D4], F32, isOutput=False)
    c0d = nc.declare_dram_parameter("c0", [1], F32, isOutput=False)
    onesd = nc.declare_dram_parameter("ones", [128], F32, isOutput=False)
    out_d = nc.declare_dram_parameter("out", [N, N], F32, isOutput=True)

    with tile.TileContext(nc) as tc:
        with ExitStack() as ctx:
            singles = ctx.enter_context(tc.tile_pool(name="singles", bufs=1))

            xT_s = singles.tile([F, N], F32)
            yT_s = singles.tile([F, N], F32)
            wxT_s = singles.tile([F, D1], F32)
            wyT_s = singles.tile([F, D1], F32)
            w2_f = singles.tile([128, 4, D2], F32)
            w3_f = singles.tile([128, 2, D3], F32)
            w2_s = singles.tile([128, 4, D2], MMDT)
            w3_s = singles.tile([128, 2, D3], MMDT)
            w4_s = singles.tile([128, D4], F32)
            w4N_s = singles.tile([D4, 128], F32)
            wc_s = singles.tile([128, 1], F32)
            c1_s = singles.tile([128, 4], F32)
            c2_s = singles.tile([128, 2], F32)
            c3_s = singles.tile([128, 1], F32)
            b4x_s = singles.tile([D4, 1], F32)
            c0_s = singles.tile([1, 1], F32)
            ones_f = singles.tile([128, 1], F32)
            ones_s = singles.tile([128, 1], MMDT)
            hx_s = singles.tile([128, 4, N], F32)
            hy_s = singles.tile([128, 4, N], F32)
            out_s = singles.tile([1, N * N], F32)

            nc.sync.dma_start(out=xT_s, in_=xT[:, :])
            nc.sync.dma_start(out=yT_s, in_=yT[:, :])
            nc.sync.dma_start(out=wxT_s, in_=wxT[:, :])
            nc.sync.dma_start(out=wyT_s, in_=wyT[:, :])
            for kc in range(4):
                nc.sync.dma_start(out=w2_f[:, kc, :], in_=w2T[kc, :, :])
            for kc in range(2):
                nc.sync.dma_start(out=w3_f[:, kc, :], in_=w3T[kc, :, :])
            nc.sync.dma_start(out=w4_s, in_=w4T[:, :])
            nc.sync.dma_start(out=w4N_s, in_=w4N[:, :])
            if SKIP_SMALL_DMA:
                for tl in (wc_s, c1_s, c2_s, c3_s, b4x_s, c0_s, ones_f):
                    nc.vector.memset(tl, 0.01)
            else:
                nc.sync.dma_start(out=wc_s[:, 0], in_=wcd[:])
                for fc in range(4):
                    nc.sync.dma_start(out=c1_s[:, fc], in_=c1d[fc, :])
                for mc in range(2):
                    nc.sync.dma_start(out=c2_s[:, mc], in_=c2d[mc, :])
                nc.sync.dma_start(out=c3_s[:, 0], in_=c3d[:])
                nc.sync.dma_start(out=b4x_s[:, 0], in_=b4x[:])
                nc.sync.dma_start(out=c0_s[0, :], in_=c0d[:])
                nc.sync.dma_start(out=ones_f[:, 0], in_=onesd[:])
            if USE_F32R:
                nc.vector.tensor_copy(out=w2_s, in_=w2_f)
                nc.vector.tensor_copy(out=w3_s, in_=w3_f)
                nc.vector.tensor_copy(out=ones_s, in_=ones_f)
            else:
                w2_s, w3_s, ones_s = w2_f, w3_f, ones_f

            # hx[f,i] (k1 folded into wx on host), hy''[f,j] (+c1).
            # Setup matmuls run in plain fp32 (tiny, and xT/yT arrive as f32).
            if SKIP_SETUP:
                nc.vector.memset(hx_s, 0.02)
                nc.vector.memset(hy_s, 0.03)
            elif True:
              with tc.tile_pool(name="psum_setup", bufs=2, space="PSUM") as pp:
                for fc in range(4):
                    ph = pp.tile([128, N], F32, tag="ph")
                    nc.tensor.matmul(
                        ph, lhsT=wxT_s[:, fc * 128:(fc + 1) * 128],
                        rhs=xT_s, start=True, stop=True)
                    nc.scalar.activation(hx_s[:, fc, :], ph, AF.Copy)
                    py_ = pp.tile([128, N], F32, tag="ph")
                    nc.tensor.matmul(
                        py_, lhsT=wyT_s[:, fc * 128:(fc + 1) * 128],
                        rhs=yT_s, start=True, stop=True)
                    nc.scalar.activation(hy_s[:, fc, :], py_, AF.Identity,
                                         bias=c1_s[:, fc:fc + 1])


            work = ctx.enter_context(tc.tile_pool(name="work", bufs=3))
            psum = ctx.enter_context(tc.tile_pool(name="psum", bufs=2,
                                                  space="PSUM"))
            psum1 = ctx.enter_context(tc.tile_pool(name="psum1", bufs=1,
                                                   space="PSUM"))

            import contextlib
            loop_cm = (tc.For_i(0, rep, 1) if rep > 1
                       else contextlib.nullcontext())
            with loop_cm:
              for t in range(NCH):
                # ---- L1: h1[f, (ii,j)] = relu(hx[f, C*t+ii] + hy[f, j]) ----
                # One fused tensor_scalar (add, then max 0) per (fc, ii);
                # the per-partition scalar is the hx column. Split between
                # DVE (2x fp32 mode) and Pool for engine balance.
                h1 = work.tile([128, 4, C * N], MMDT, tag="h1")
                if skip_l1:
                    nc.vector.tensor_scalar(
                        out=h1[:, :, :].rearrange(
                            "p a (b c) -> p a b c", b=C),
                        in0=hy_s.unsqueeze(2).broadcast_to([128, 4, C, N]),
                        scalar1=hx_s[:, 0, 0:1],
                        scalar2=0.0,
                        op0=ALU.add, op1=ALU.max)
                else:
                    for fc in range(4):
                        for ii in range(C):
                            idx = fc * C + ii
                            eng = (nc.vector if idx in _l1_dve
                                   else nc.gpsimd)
                            eng.tensor_scalar(
                                out=h1[:, fc, ii * N:(ii + 1) * N],
                                in0=hy_s[:, fc, :],
                                scalar1=hx_s[:, fc, C * t + ii:C * t + ii + 1],
                                scalar2=0.0,
                                op0=ALU.add, op1=ALU.max)

                if _skip_mlp:
                    nc.vector.tensor_copy(
                        out=out_s[0:1, C * N * t:C * N * (t + 1)],
                        in_=h1[0:1, 0, :].bitcast(F32))
                    continue
                # ---- L2: h2 = relu(W2' h1 + c2) ----
                h2 = work.tile([128, 2, C * N], MMDT, tag="h2")
                for mc in range(2):
                    p2 = psum.tile([128, C * N], F32, tag="p2")
                    for kc in range(4):
                        nc.tensor.matmul(
                            p2,
                            lhsT=w2_s[:, kc, mc * 128:(mc + 1) * 128],
                            rhs=h1[:, kc, :],
                            start=(kc == 0), stop=(kc == 3))
                    nc.scalar.activation(h2[:, mc, :], p2, AF.Relu,
                                         bias=c2_s[:, mc:mc + 1])

                # ---- L3: h3 = relu(W3' h2 + c3) ----
                h3 = work.tile([128, C * N], MMDT, tag="h3")
                p3 = psum.tile([128, C * N], F32, tag="p3")
                for kc in range(2):
                    nc.tensor.matmul(
                        p3, lhsT=w3_s[:, kc, :], rhs=h2[:, kc, :],
                        start=(kc == 0), stop=(kc == 1))
                nc.scalar.activation(h3, p3, AF.Relu, bias=c3_s[:, 0:1])
                h3f = h3.bitcast(F32)

                # ---- head ----
                if _skip_head:
                    nc.vector.tensor_copy(
                        out=out_s[0:1, C * N * t:C * N * (t + 1)],
                        in_=h3f[0:1, :])
                    continue
                q = work.tile([128, C], F32, tag="q")
                nc.vector.tensor_reduce(
                    out=q, in_=h3f.rearrange("p (a b) -> p a b", a=C),
                    axis=AX.X, op=ALU.add)

                ps_s = psum1.tile([D4, C], F32, tag="ss")
                nc.tensor.matmul(ps_s, lhsT=w4_s, rhs=q,
                                 start=True, stop=True)
                s_sb = work.tile([D4, C], F32, tag="s")
                nc.scalar.activation(s_sb, ps_s, AF.Identity, bias=b4x_s)

                ps_v = psum1.tile([128, C], F32, tag="sv")
                nc.tensor.matmul(ps_v, lhsT=w4N_s, rhs=s_sb,
                                 start=True, stop=True)
                v_sb = work.tile([128, C], F32, tag="v")
                nc.scalar.activation(v_sb, ps_v, AF.Copy)

                ps_c = psum1.tile([1, C], F32, tag="sc")
                nc.tensor.matmul(ps_c, lhsT=wc_s, rhs=q,
                                 start=True, stop=True)
                c_sb = work.tile([1, C], F32, tag="c")
                nc.scalar.activation(c_sb, ps_c, AF.Identity,
                                     bias=c0_s[0:1, 0:1])

                em = work.tile([128, C * N], MMDT, tag="em")
                for ii in range(C):
                    nc.vector.tensor_scalar(
                        out=em[:, ii * N:(ii + 1) * N],
                        in0=h3f[:, ii * N:(ii + 1) * N],
                        scalar1=v_sb[:, ii:ii + 1],
                        scalar2=None,
                        op0=ALU.mult)

                pf = psum1.tile([1, C * N], F32, tag="pf")
                nc.tensor.matmul(pf, lhsT=ones_s, rhs=em,
                                 start=True, stop=True)
                nc.vector.tensor_tensor(
                    out=out_s[0:1, C * N * t:C * N * (t + 1)].rearrange(
                        "p (a b) -> p a b", a=C),
                    in0=pf.rearrange("p (a b) -> p a b", a=C),
                    in1=c_sb.unsqueeze(2).broadcast_to([1, C, N]),
                    op=ALU.add)

            nc.sync.dma_start(
                out=out_d[:, :].unsqueeze(0),
                in_=out_s[0:1, :].rearrange("p (a b) -> p a b", a=N))
    nc.finalize()
    return nc


_NC_CACHE = None


def _get_nc():
    global _NC_CACHE
    if _NC_CACHE is None:
        _NC_CACHE = build_module()
    return _NC_CACHE


def host_prep(inputs):
    """Fold the BatchNorm affines into weights/biases; pre-transpose
    everything into the device layouts. Returns the per-core input maps."""
    f32 = np.float32
    x = np.asarray(inputs["x"], f32)
    y = np.asarray(inputs["y"], f32)
    w1, b1 = np.asarray(inputs["w1"], f32), np.asarray(inputs["b1"], f32)
    w2, b2 = np.asarray(inputs["w2"], f32), np.asarray(inputs["b2"], f32)
    w3, b3 = np.asarray(inputs["w3"], f32), np.asarray(inputs["b3"], f32)
    w4, b4 = np.asarray(inputs["w4"], f32), np.asarray(inputs["b4"], f32)

    k1 = inputs["g1"] / np.sqrt(inputs["v1"] + EPS)
    c1 = k1 * (b1 - inputs["m1"]) + inputs["be1"]
    k2 = inputs["g2"] / np.sqrt(inputs["v2"] + EPS)
    c2 = k2 * (b2 - inputs["m2"]) + inputs["be2"]
    k3 = inputs["g3"] / np.sqrt(inputs["v3"] + EPS)
    c3 = k3 * (b3 - inputs["m3"]) + inputs["be3"]

    wx = w1[:, :F] * k1[:, None]          # (512, 128)
    wy = w1[:, F:] * k1[:, None]
    w2f = w2 * k2[:, None]                # (256, 512)
    w3f = w3 * k3[:, None]                # (128, 256)

    shared = {
        "wxT": np.ascontiguousarray(wx.T, f32),                    # (128, 512)
        "wyT": np.ascontiguousarray(wy.T, f32),
        "w2T": np.ascontiguousarray(w2f.T.reshape(4, 128, D2), f32),
        "w3T": np.ascontiguousarray(w3f.T.reshape(2, 128, D3), f32),
        "w4T": np.ascontiguousarray(w4.T, f32),                    # (128, 64)
        "w4N": np.ascontiguousarray(w4, f32),                      # (64, 128)
        "wc": np.ascontiguousarray(w4.T @ b4, f32),                # (128,)
        "c1": np.ascontiguousarray(c1.reshape(4, 128), f32),
        "c2": np.ascontiguousarray(c2.reshape(2, 128), f32),
        "c3": np.ascontiguousarray(c3, f32),
        "b4x": np.ascontiguousarray(N * b4, f32),                  # ny*b4
        "c0": np.asarray([N * float(b4 @ b4)], f32),               # ny*|b4|^2
        "ones": np.ones(128, f32),
    }
    in_maps = []
    for b in range(B):
        m = dict(shared)
        m["xT"] = np.ascontiguousarray(x[b].T, f32)
        m["yT"] = np.ascontiguousarray(y[b].T, f32)
        in_maps.append(m)
    return in_maps


def kernel(**inputs):
    nc = _get_nc()
    in_maps = host_prep(inputs)
    res = run_bass_kernel_spmd(nc, in_maps, list(range(B)))
    out = np.stack([res.results[b]["out"] for b in range(B)], axis=0)
    return out.astype(np.float32)


# revision 24
# speedup vs baseline: 3.4084x; 3.3269x over previous
"""Trainium2 Bass kernel for nn_DKOKernel (dense pairwise MLP + PSD head).

Math (per batch b):
  hx[f,i] = sum_c wx[f,c] x[b,i,c];  hy[f,j] = sum_c wy[f,c] y[b,j,c]
  h1 = relu(bn1(hx_i + hy_j + b1))          (512)
  h2 = relu(bn2(W2 h1 + b2))                (256)
  h3 = relu(bn3(W3 h2 + b3))                (128)
  e  = W4 h3 + b4                           (64)
  s[b,i,:] = sum_j e[b,i,j,:]
  out[b,i,j] = sum_k e[b,i,j,k] * s[b,i,k]

Sharding: data-parallel over batch — core b handles batch b (B=8 = n_cores).
All BatchNorm affine factors are folded into the weights/biases on the host.

Head algebra (avoids materializing e):
  q_i  = sum_j h3_ij                    (per-i, via free-dim reduce)
  s_i  = W4 q_i + ny*b4                 (tiny N=C matmul, fp32)
  v_i  = W4^T s_i                       (tiny N=C matmul, fp32)
  c_i  = wc.q_i + ny*|b4|^2             (wc = W4^T b4, tiny N=C matmul, fp32)
  out[i,j] = h3_ij . v_i + c_i          (elementwise mul + ones-matmul + add)

Device-side layout: features on partitions, (i-block, j) pairs on the free
dim. C=4 i-rows per chunk => free dim 512 per matmul (fp32 PSUM bank max and
the ap_size>=256 threshold where fp32r matmuls run 1 cycle/row).

The big matmuls (L2, L3, ones-reduce) run in float32r (TF32-like, ~1.6e-4
relative per matmul, 4x faster than fp32). Their operands must be produced
as float32r: the weights are DMA-loaded into f32r tiles, the activations are
written as f32r by the producing DVE/Pool/ACT ops.
"""

import numpy as np
from contextlib import ExitStack

import concourse.bacc as bacc
import concourse.tile as tile
from concourse import mybir
from concourse.bass_utils import run_bass_kernel_spmd

F32 = mybir.dt.float32
F32R = mybir.dt.float32r
AF = mybir.ActivationFunctionType
ALU = mybir.AluOpType
AX = mybir.AxisListType

EPS = 1e-5
B = 8
N = 128          # nx == ny
F = 128          # input feature dim
D1, D2, D3, D4 = 512, 256, 128, 64
C = 4            # i-rows per chunk -> free dim C*N = 512
import os
NCH = int(os.environ.get('NCH_OVERRIDE', N // C))
SKIP_HEAD = os.environ.get('SKIP_HEAD', '0') == '1'
SKIP_SMALL_DMA = os.environ.get('SKIP_SMALL_DMA', '0') == '1'
SKIP_SETUP = os.environ.get('SKIP_SETUP', '0') == '1'
SKIP_MLP = os.environ.get('SKIP_MLP', '0') == '1'
REPEAT = int(os.environ.get('REPEAT', '1'))

USE_F32R = True
MMDT = F32R if USE_F32R else F32
# L1 engine assignment: one char per (fc, ii) quarter-op.
# V = DVE fused tensor_scalar, A = ACT relu+bias, P = Pool TT pair
# (P must cover whole fc groups of 4).
L1_ASSIGN = "VVVV" "VVVV" "VAAA" "AAAA"


def build_module(repeat=None, skip_l1=False, skip_mlp=None,
                 skip_head=None, l1_alldve=False):
    rep = REPEAT if repeat is None else repeat
    _skip_mlp = SKIP_MLP if skip_mlp is None else skip_mlp
    _skip_head = SKIP_HEAD if skip_head is None else skip_head
    _l1_assign = "V" * 16 if l1_alldve else L1_ASSIGN
    nc = bacc.Bacc()

    xT = nc.declare_dram_parameter("xT", [F, N], F32, isOutput=False)
    yT = nc.declare_dram_parameter("yT", [F, N], F32, isOutput=False)
    wxT = nc.declare_dram_parameter("wxT", [F, D1], F32, isOutput=False)
    wyT = nc.declare_dram_parameter("wyT", [F, D1], F32, isOutput=False)
    w2T = nc.declare_dram_parameter("w2T", [4, 128, D2], F32, isOutput=False)
    w3T = nc.declare_dram_parameter("w3T", [2, 128, D3], F32, isOutput=False)
    w4T = nc.declare_dram_parameter("w4T", [128, D4], F32, isOutput=False)
    w4N = nc.declare_dram_parameter("w4N", [D4, 128], F32, isOutput=False)
    wcd = nc.declare_dram_parameter("wc", [128], F32, isOutput=False)
    c1d = nc.declare_dram_parameter("c1", [4, 128], F32, isOutput=False)
    c2d = nc.declare_dram_parameter("c2", [2, 128], F32, isOutput=False)
    c3d = nc.declare_dram_parameter("c3", [128], F32, isOutput=False)
    b4x = nc.declare_dram_parameter("b4x", [D4], F32, isOutput=False)
    c0d = nc.declare_dram_parameter("c0", [1], F32, isOutput=False)
    onesd = nc.declare_dram_parameter("ones", [128], F32, isOutput=False)
    out_d = nc.declare_dram_parameter("out", [N, N], F32, isOutput=True)

    with tile.TileContext(nc) as tc:
        with ExitStack() as ctx:
            singles = ctx.enter_context(tc.tile_pool(name="singles", bufs=1))

            xT_s = singles.tile([F, N], F32)
            yT_s = singles.tile([F, N], F32)
            wxT_s = singles.tile([F, D1], F32)
            wyT_s = singles.tile([F, D1], F32)
            w2_f = singles.tile([128, 4, D2], F32)
            w3_f = singles.tile([128, 2, D3], F32)
            w2_s = singles.tile([128, 4, D2], MMDT)
            w3_s = singles.tile([128, 2, D3], MMDT)
            w4_s = singles.tile([128, D4], F32)
            w4N_s = singles.tile([D4, 128], F32)
            wc_s = singles.tile([128, 1], F32)
            c1_s = singles.tile([128, 4], F32)
            c2_s = singles.tile([128, 2], F32)
            c3_s = singles.tile([128, 1], F32)
            b4x_s = singles.tile([D4, 1], F32)
            c0_s = singles.tile([1, 1], F32)
            ones_f = singles.tile([128, 1], F32)
            ones_s = singles.tile([128, 1], MMDT)
            hx_s = singles.tile([128, 4, N], F32)
            hy_s = singles.tile([128, 4, N], F32)
            out_s = singles.tile([1, N * N], F32)
            zero_s = singles.tile([128, 1], F32)

            nc.sync.dma_start(out=xT_s, in_=xT[:, :])
            nc.sync.dma_start(out=yT_s, in_=yT[:, :])
            nc.sync.dma_start(out=wxT_s, in_=wxT[:, :])
            nc.sync.dma_start(out=wyT_s, in_=wyT[:, :])
            for kc in range(4):
                nc.sync.dma_start(out=w2_f[:, kc, :], in_=w2T[kc, :, :])
            for kc in range(2):
                nc.sync.dma_start(out=w3_f[:, kc, :], in_=w3T[kc, :, :])
            nc.sync.dma_start(out=w4_s, in_=w4T[:, :])
            nc.sync.dma_start(out=w4N_s, in_=w4N[:, :])
            if SKIP_SMALL_DMA:
                for tl in (wc_s, c1_s, c2_s, c3_s, b4x_s, c0_s, ones_f):
                    nc.vector.memset(tl, 0.01)
            else:
                nc.sync.dma_start(out=wc_s[:, 0], in_=wcd[:])
                for fc in range(4):
                    nc.sync.dma_start(out=c1_s[:, fc], in_=c1d[fc, :])
                for mc in range(2):
                    nc.sync.dma_start(out=c2_s[:, mc], in_=c2d[mc, :])
                nc.sync.dma_start(out=c3_s[:, 0], in_=c3d[:])
                nc.sync.dma_start(out=b4x_s[:, 0], in_=b4x[:])
                nc.sync.dma_start(out=c0_s[0, :], in_=c0d[:])
                nc.sync.dma_start(out=ones_f[:, 0], in_=onesd[:])
            nc.vector.memset(zero_s, 0.0)
            if USE_F32R:
                nc.vector.tensor_copy(out=w2_s, in_=w2_f)
                nc.vector.tensor_copy(out=w3_s, in_=w3_f)
                nc.vector.tensor_copy(out=ones_s, in_=ones_f)
            else:
                w2_s, w3_s, ones_s = w2_f, w3_f, ones_f

            # hx[f,i] (k1 folded into wx on host), hy''[f,j] (+c1).
            # Setup matmuls run in plain fp32 (tiny, and xT/yT arrive as f32).
            if SKIP_SETUP:
                nc.vector.memset(hx_s, 0.02)
                nc.vector.memset(hy_s, 0.03)
            elif True:
              with tc.tile_pool(name="psum_setup", bufs=2, space="PSUM") as pp:
                for fc in range(4):
                    ph = pp.tile([128, N], F32, tag="ph")
                    nc.tensor.matmul(
                        ph, lhsT=wxT_s[:, fc * 128:(fc + 1) * 128],
                        rhs=xT_s, start=True, stop=True)
                    nc.scalar.activation(hx_s[:, fc, :], ph, AF.Copy)
                    py_ = pp.tile([128, N], F32, tag="ph")
                    nc.tensor.matmul(
                        py_, lhsT=wyT_s[:, fc * 128:(fc + 1) * 128],
                        rhs=yT_s, start=True, stop=True)
                    nc.scalar.activation(hy_s[:, fc, :], py_, AF.Identity,
                                         bias=c1_s[:, fc:fc + 1])


            work = ctx.enter_context(tc.tile_pool(name="work", bufs=3))
            psum = ctx.enter_context(tc.tile_pool(name="psum", bufs=2,
                                                  space="PSUM"))
            psum1 = ctx.enter_context(tc.tile_pool(name="psum1", bufs=1,
                                                   space="PSUM"))

            import contextlib
            loop_cm = (tc.For_i(0, rep, 1) if rep > 1
                       else contextlib.nullcontext())
            with loop_cm:
              for t in range(NCH):
                # ---- L1: h1[f, (ii,j)] = relu(hx[f, C*t+ii] + hy[f, j]) ----
                # One fused tensor_scalar (add, then max 0) per (fc, ii);
                # the per-partition scalar is the hx column. Split between
                # DVE (2x fp32 mode) and Pool for engine balance.
                h1 = work.tile([128, 4, C * N], MMDT, tag="h1")
                if skip_l1:
                    nc.vector.tensor_scalar(
                        out=h1[:, :, :].rearrange(
                            "p a (b c) -> p a b c", b=C),
                        in0=hy_s.unsqueeze(2).broadcast_to([128, 4, C, N]),
                        scalar1=hx_s[:, 0, 0:1],
                        scalar2=0.0,
                        op0=ALU.add, op1=ALU.max)
                else:
                    h1p = work.tile([128, 4, C * N], F32, tag="h1p")
                    for fc in range(4):
                        modes = _l1_assign[fc * C:(fc + 1) * C]
                        if modes == "PPPP":
                            nc.gpsimd.tensor_tensor(
                                out=h1p[:, fc, :].rearrange(
                                    "p (a b) -> p a b", a=C),
                                in0=hy_s[:, fc, :].unsqueeze(1)
                                    .broadcast_to([128, C, N]),
                                in1=hx_s[:, fc, C * t:C * t + C].unsqueeze(2)
                                    .broadcast_to([128, C, N]),
                                op=ALU.add)
                            nc.gpsimd.tensor_tensor(
                                out=h1[:, fc, :],
                                in0=h1p[:, fc, :],
                                in1=zero_s.broadcast_to([128, C * N]),
                                op=ALU.max)
                            continue
                        for ii in range(C):
                            m = modes[ii]
                            sl = slice(ii * N, (ii + 1) * N)
                            xc = hx_s[:, fc, C * t + ii:C * t + ii + 1]
                            if m == "V":
                                nc.vector.tensor_scalar(
                                    out=h1[:, fc, sl],
                                    in0=hy_s[:, fc, :],
                                    scalar1=xc, scalar2=0.0,
                                    op0=ALU.add, op1=ALU.max)
                            else:
                                nc.scalar.activation(
                                    h1[:, fc, sl], hy_s[:, fc, :],
                                    AF.Relu, bias=xc)

                if _skip_mlp:
                    nc.vector.tensor_copy(
                        out=out_s[0:1, C * N * t:C * N * (t + 1)],
                        in_=h1[0:1, 0, :].bitcast(F32))
                    continue
                # ---- L2: h2 = relu(W2' h1 + c2) ----
                h2 = work.tile([128, 2, C * N], MMDT, tag="h2")
                for mc in range(2):
                    p2 = psum.tile([128, C * N], F32, tag="p2")
                    for kc in range(4):
                        nc.tensor.matmul(
                            p2,
                            lhsT=w2_s[:, kc, mc * 128:(mc + 1) * 128],
                            rhs=h1[:, kc, :],
                            start=(kc == 0), stop=(kc == 3))
                    nc.scalar.activation(h2[:, mc, :], p2, AF.Relu,
                                         bias=c2_s[:, mc:mc + 1])

                # ---- L3: h3 = relu(W3' h2 + c3) ----
                h3 = work.tile([128, C * N], MMDT, tag="h3")
                p3 = psum.tile([128, C * N], F32, tag="p3")
                for kc in range(2):
                    nc.tensor.matmul(
                        p3, lhsT=w3_s[:, kc, :], rhs=h2[:, kc, :],
                        start=(kc == 0), stop=(kc == 1))
                nc.scalar.activation(h3, p3, AF.Relu, bias=c3_s[:, 0:1])
                h3f = h3.bitcast(F32)

                # ---- head ----
                if _skip_head:
                    nc.vector.tensor_copy(
                        out=out_s[0:1, C * N * t:C * N * (t + 1)],
                        in_=h3f[0:1, :])
                    continue
                q = work.tile([128, C], F32, tag="q")
                nc.vector.tensor_reduce(
                    out=q, in_=h3f.rearrange("p (a b) -> p a b", a=C),
                    axis=AX.X, op=ALU.add)

                ps_s = psum1.tile([D4, C], F32, tag="ss")
                nc.tensor.matmul(ps_s, lhsT=w4_s, rhs=q,
                                 start=True, stop=True)
                s_sb = work.tile([D4, C], F32, tag="s")
                nc.scalar.activation(s_sb, ps_s, AF.Identity, bias=b4x_s)

                ps_v = psum1.tile([128, C], F32, tag="sv")
                nc.tensor.matmul(ps_v, lhsT=w4N_s, rhs=s_sb,
                                 start=True, stop=True)
                v_sb = work.tile([128, C], F32, tag="v")
                nc.scalar.activation(v_sb, ps_v, AF.Copy)

                ps_c = psum1.tile([1, C], F32, tag="sc")
                nc.tensor.matmul(ps_c, lhsT=wc_s, rhs=q,
                                 start=True, stop=True)
                c_sb = work.tile([1, C], F32, tag="c")
                nc.scalar.activation(c_sb, ps_c, AF.Identity,
                                     bias=c0_s[0:1, 0:1])

                em = work.tile([128, C * N], MMDT, tag="em")
                nc.vector.tensor_tensor(
                    out=em.rearrange("p (a b) -> p a b", a=C),
                    in0=h3f.rearrange("p (a b) -> p a b", a=C),
                    in1=v_sb.unsqueeze(2).broadcast_to([128, C, N]),
                    op=ALU.mult)

                pf = psum1.tile([1, C * N], F32, tag="pf")
                nc.tensor.matmul(pf, lhsT=ones_s, rhs=em,
                                 start=True, stop=True)
                nc.vector.tensor_tensor(
                    out=out_s[0:1, C * N * t:C * N * (t + 1)].rearrange(
                        "p (a b) -> p a b", a=C),
                    in0=pf.rearrange("p (a b) -> p a b", a=C),
                    in1=c_sb.unsqueeze(2).broadcast_to([1, C, N]),
                    op=ALU.add)

            nc.sync.dma_start(
                out=out_d[:, :].unsqueeze(0),
                in_=out_s[0:1, :].rearrange("p (a b) -> p a b", a=N))
    nc.finalize()
    return nc


_NC_CACHE = None


def _get_nc():
    global _NC_CACHE
    if _NC_CACHE is None:
        _NC_CACHE = build_module()
    return _NC_CACHE


def host_prep(inputs):
    """Fold the BatchNorm affines into weights/biases; pre-transpose
    everything into the device layouts. Returns the per-core input maps."""
    f32 = np.float32
    x = np.asarray(inputs["x"], f32)
    y = np.asarray(inputs["y"], f32)
    w1, b1 = np.asarray(inputs["w1"], f32), np.asarray(inputs["b1"], f32)
    w2, b2 = np.asarray(inputs["w2"], f32), np.asarray(inputs["b2"], f32)
    w3, b3 = np.asarray(inputs["w3"], f32), np.asarray(inputs["b3"], f32)
    w4, b4 = np.asarray(inputs["w4"], f32), np.asarray(inputs["b4"], f32)

    k1 = inputs["g1"] / np.sqrt(inputs["v1"] + EPS)
    c1 = k1 * (b1 - inputs["m1"]) + inputs["be1"]
    k2 = inputs["g2"] / np.sqrt(inputs["v2"] + EPS)
    c2 = k2 * (b2 - inputs["m2"]) + inputs["be2"]
    k3 = inputs["g3"] / np.sqrt(inputs["v3"] + EPS)
    c3 = k3 * (b3 - inputs["m3"]) + inputs["be3"]

    wx = w1[:, :F] * k1[:, None]          # (512, 128)
    wy = w1[:, F:] * k1[:, None]
    w2f = w2 * k2[:, None]                # (256, 512)
    w3f = w3 * k3[:, None]                # (128, 256)

    shared = {
        "wxT": np.ascontiguousarray(wx.T, f32),                    # (128, 512)
        "wyT": np.ascontiguousarray(wy.T, f32),
        "w2T": np.ascontiguousarray(w2f.T.reshape(4, 128, D2), f32),
        "w3T": np.ascontiguousarray(w3f.T.reshape(2, 128, D3), f32),
        "w4T": np.ascontiguousarray(w4.T, f32),                    # (128, 64)
        "w4N": np.ascontiguousarray(w4, f32),                      # (64, 128)
        "wc": np.ascontiguousarray(w4.T @ b4, f32),                # (128,)
        "c1": np.ascontiguousarray(c1.reshape(4, 128), f32),
        "c2": np.ascontiguousarray(c2.reshape(2, 128), f32),
        "c3": np.ascontiguousarray(c3, f32),
        "b4x": np.ascontiguousarray(N * b4, f32),                  # ny*b4
        "c0": np.asarray([N * float(b4 @ b4)], f32),               # ny*|b4|^2
        "ones": np.ones(128, f32),
    }
    in_maps = []
    for b in range(B):
        m = dict(shared)
        m["xT"] = np.ascontiguousarray(x[b].T, f32)
        m["yT"] = np.ascontiguousarray(y[b].T, f32)
        in_maps.append(m)
    return in_maps


def kernel(**inputs):
    nc = _get_nc()
    in_maps = host_prep(inputs)
    res = run_bass_kernel_spmd(nc, in_maps, list(range(B)))
    out = np.stack([res.results[b]["out"] for b in range(B)], axis=0)
    return out.astype(np.float32)
